# revision 1
# baseline (speedup 1.0000x reference)
"""Causal self-attention (b=2, n=2048, d=1024, 16 heads) on 8 NeuronCores.

Sharding: core c handles batch b = c // 4 and head group g = c % 4
(heads 4g..4g+3).  qkv weights column-sharded, proj weights row-sharded
(Megatron); each core emits a partial [2048, 1024] proj output and the
host sums the 4 partials per batch (b_proj also added host-side).

On-device layout (per core, all matmuls in float32r):
  xT   [1024, 2048]  x[b] transposed (host prep)
  qT,kT feature-major [128p, pair, 2048] (2 head pairs, 64-dim heads
        stacked on partitions) -> QK^T computed as S^T[k, q] with two
        K=64 matmuls packed in the PE array via base-partition 0/64.
  V     token-major with a fused ones column per head ([V|1]) so the
        AV matmul also produces the softmax denominator (row 64).
  exp   on ACT (scale=1/8 fused), causal mask = multiplicative f32 tile
        built on gpsimd; only lower-triangle blocks computed.
  normalize: reciprocal on DVE + PE ones-broadcast + DVE multiply.

Emission order is tuned so PE never starves: per token-quarter we do
qkv m-tiles, V blocks, the previous quarter's output projection, then
the attention i-loop with QK running 4 blocks ahead of AV.
"""
import sys

sys.path.insert(0, "/opt/trn_rl_repo")

import numpy as np

import concourse.bass as bass  # noqa: F401
import concourse.mybir as mybir
import concourse.tile as tile
from concourse import bacc
from concourse.bass_utils import run_bass_kernel_spmd

F32 = mybir.dt.float32
F32R = mybir.dt.float32r
Exp = mybir.ActivationFunctionType.Exp
Ident = mybir.ActivationFunctionType.Identity

B = 2
N = 2048
D = 1024
NH = 16
HD = 64
NCORES = 8
GROUPS = 4                # head groups (cores per batch)
HPC = NH // GROUPS        # heads per core = 4
PAIRS = HPC // 2          # head pairs per core = 2
QS = 512                  # q_super width
NQS = N // QS             # 4
NB = N // 128             # 16 token blocks
CCH = D // 128            # 8 contraction chunks

_CACHE = {}


def _build():
    nc = bacc.Bacc("TRN2", target_bir_lowering=False, debug=False,
                   num_devices=NCORES)
    xT = nc.dram_tensor("xT", [D, N], F32R, kind="ExternalInput").ap()
    W = nc.dram_tensor("W", [D, 768], F32R, kind="ExternalInput").ap()
    Wp = nc.dram_tensor("Wp", [256, D], F32R, kind="ExternalInput").ap()
    biasqk = nc.dram_tensor("biasqk", [128, 4], F32, kind="ExternalInput").ap()
    vbias = nc.dram_tensor("vbias", [128, 256], F32, kind="ExternalInput").ap()
    ones64D = nc.dram_tensor("ones64D", [1, 64], F32R, kind="ExternalInput").ap()
    y = nc.dram_tensor("y", [N, D], F32, kind="ExternalOutput").ap()

    with tile.TileContext(nc) as tc:
        with (
            tc.tile_pool(name="persist", bufs=1) as pp,
            tc.tile_pool(name="xtq_pool", bufs=2) as xtq_pool,
            tc.tile_pool(name="et_pool", bufs=8) as et_pool,
            tc.tile_pool(name="work", bufs=3) as work,
            tc.tile_pool(name="ysb_pool", bufs=6) as ysb_pool,
            tc.tile_pool(name="mm", bufs=2, space="PSUM") as mm,
            tc.tile_pool(name="spool", bufs=2, space="PSUM") as spool,
            tc.tile_pool(name="opool", bufs=2, space="PSUM") as opool,
        ):
            # ---- persistent tiles ----
            W_sb = pp.tile([128, CCH, 768], F32R)
            Wp_sb = pp.tile([128, 2, D], F32R)
            bqk_sb = pp.tile([128, 4], F32)
            vbias_sb = pp.tile([128, 256], F32)
            ones64 = pp.tile([1, 64], F32R)
            qT = pp.tile([128, PAIRS, N], F32R)
            kT = pp.tile([128, PAIRS, N], F32R)
            onT = pp.tile([128, PAIRS, N], F32R)
            vaug = pp.tile([128, NB, HPC * 65], F32R)
            vaug_h = vaug.rearrange("p b (h c) -> p b h c", c=65)
            masks = pp.tile([128, 4, QS], F32)

            W_r = W.rearrange("(c p) f -> p c f", p=128)
            Wp_r = Wp.rearrange("(c p) f -> p c f", p=128)
            xT_r = xT.rearrange("(c p) n -> p c n", p=128)
            y_r = y.rearrange("(t p) f -> t p f", p=128)

            # causal masks on gpsimd (off the DMA critical path):
            # masks[p, t, q] = 1.0 iff q - p - 128*t >= 0
            nc.gpsimd.memset(masks[:], 1.0)
            for t in range(4):
                nc.gpsimd.affine_select(
                    out=masks[:, t, :],
                    in_=masks[:, t, :],
                    compare_op=mybir.AluOpType.is_ge,
                    fill=0.0,
                    base=-128 * t,
                    pattern=[[1, QS]],
                    channel_multiplier=-1,
                )

            pending_norm = []

            def emit_norm(bc_on_dve=False):
                """normalize deferred (j, hp, osb) entries: overlap the DVE
                reciprocal chain with the next quarter's PE work.
                bc_on_dve: use DVE for the broadcast copy when flushing into
                an ACT-busy (exp-heavy) window."""
                while pending_norm:
                    j, hp, osb = pending_norm.pop(0)
                    if osb[0].space == bass.MemorySpace.PSUM and j < NQS - 1:
                        # deferred drain: copy here so it queues AFTER the
                        # m-tile copybacks on DVE (slot-recycling order)
                        o_ps = osb
                        osb = {}
                        for h in range(2):
                            osb[h] = work.tile([65, QS], F32, tag="osb",
                                               bufs=4, name=f"osbd{j}{hp}{h}")
                            nc.vector.tensor_copy(osb[h][:], o_ps[h][:])
                    for h in range(2):
                        pb = 64 * h
                        recip = work.tile([1, QS], F32R, tag="recip",
                                          name=f"r{j}{hp}{h}")
                        with nc.allow_low_precision("f32r recip for PE bcast"):
                            nc.vector.reciprocal(recip[:], osb[h][64:65, :])
                        bc_ps = mm.tile([64, QS], F32, tag="mm",
                                        name=f"bc{j}{hp}{h}")
                        nc.tensor.matmul(bc_ps[:], ones64[:], recip[:],
                                         start=True, stop=True)
                        bc_sb = work.tile([64, QS], F32, tag="bc_sb",
                                          name=f"bs{j}{hp}{h}")
                        if bc_on_dve:
                            nc.vector.tensor_copy(bc_sb[:], bc_ps[:])
                        else:
                            nc.scalar.copy(bc_sb[:], bc_ps[:])
                        nc.vector.tensor_mul(
                            onT[pb : pb + 64, hp, QS * j : QS * (j + 1)],
                            osb[h][0:64, :],
                            bc_sb[:],
                        )

            def make_proj_units(jj, tail=False):
                """output projection for quarter jj as one closure per
                (block, half) unit, so units can be interleaved into the
                ACT-bound attention i-loop as PE fillers"""
                def unit(blk, nh):
                    def emit():
                        tb = 4 * jj + blk
                        yps = mm.tile([128, QS], F32, tag="mm",
                                      name=f"y{tb}{nh}")
                        for c in range(2):
                            nc.tensor.matmul(
                                yps[:],
                                onT[:, c, 128 * tb : 128 * (tb + 1)],
                                Wp_sb[:, c, QS * nh : QS * (nh + 1)],
                                start=(c == 0),
                                stop=(c == 1),
                            )
                        ysb = ysb_pool.tile([128, QS], F32, tag="ysb",
                                            name=f"ysb{tb}{nh}")
                        # tail: ACT is idle — alternate copy engines
                        if tail and (blk + nh) % 2 == 1:
                            nc.scalar.copy(ysb[:], yps[:])
                        else:
                            nc.vector.tensor_copy(ysb[:], yps[:])
                        nc.sync.dma_start(
                            y_r[tb][:, QS * nh : QS * (nh + 1)], ysb[:]
                        )
                    return emit
                return [unit(blk, nh) for blk in range(4) for nh in range(2)]

            def emit_proj(jj, tail=False):
                for u in make_proj_units(jj, tail):
                    u()

            def fetch_xq(q):
                t0, t1 = QS * q, QS * (q + 1)
                xq = xtq_pool.tile([128, CCH, QS], F32R, tag="xq",
                                   name=f"xq{q}")
                for ci in range(CCH):
                    nc.sync.dma_start(xq[:, ci, :], xT_r[:, ci, t0:t1])
                return xq

            next_xq = None
            for qtr in range(NQS):
                ts, te = QS * qtr, QS * (qtr + 1)
                j = qtr

                # ---- input DMAs, ordered by first consumption ----
                if qtr == 0:
                    xq = xtq_pool.tile([128, CCH, QS], F32R, tag="xq",
                                       name="xq0")
                    for ci in range(CCH):
                        nc.sync.dma_start(W_sb[:, ci, 0:512], W_r[:, ci, 0:512])
                        nc.sync.dma_start(xq[:, ci, :], xT_r[:, ci, ts:te])
                    nc.sync.dma_start(bqk_sb[:], biasqk)
                    # ones columns of [V|1] via DVE (0*x + 1) — avoids a
                    # descriptor-heavy 64-column scatter DMA
                    nc.vector.tensor_scalar(
                        out=vaug_h[:, :, :, 64],
                        in0=W_sb[:, 0, 0:64].rearrange(
                            "p (a b) -> p a b", b=HPC
                        ),
                        scalar1=0.0,
                        scalar2=1.0,
                        op0=mybir.AluOpType.mult,
                        op1=mybir.AluOpType.add,
                    )
                    # v-columns are consumed late (V runs inside hp0's
                    # attention) — keep them off the critical qk prefix
                    for ci in range(CCH):
                        nc.sync.dma_start(W_sb[:, ci, 512:768],
                                          W_r[:, ci, 512:768])
                    nc.sync.dma_start(vbias_sb[:], vbias)
                    next_xq = fetch_xq(1)
                    nc.sync.dma_start(ones64[:], ones64D)
                    for c in range(2):
                        nc.sync.dma_start(Wp_sb[:, c, :], Wp_r[:, c, :])
                else:
                    xq = next_xq
                    if qtr + 1 < NQS:
                        next_xq = fetch_xq(qtr + 1)

                # ---- qkv projection: q/k feature-major m-tiles ----
                # pair-major halves so pair 0's q AND k finish first;
                # chunk-outer so quarter 0 consumes x chunks as they arrive
                for half in ((0, 2), (1, 3)):
                    ps = {
                        m: mm.tile([128, QS], F32, tag="mm", name=f"qk{qtr}{m}")
                        for m in half
                    }
                    for ci in range(CCH):
                        for m in half:
                            nc.tensor.matmul(
                                ps[m][:],
                                W_sb[:, ci, 128 * m : 128 * (m + 1)],
                                xq[:, ci, :],
                                start=(ci == 0),
                                stop=(ci == CCH - 1),
                            )
                    for m in half:
                        dst = qT if m < 2 else kT
                        nc.vector.tensor_scalar_add(
                            dst[:, m % 2, ts:te], ps[m][:], bqk_sb[:, m : m + 1]
                        )

                # ---- V token-major (with bias) into [V|1] slots ----
                # as filler units: V(blk) is only consumed by the diagonal
                # AVs, which sit in hp0's drain — so V can interleave into
                # the ACT-bound i-loop
                def make_v_units(q=qtr, xq_=xq):
                    def unit(blk):
                        def emit():
                            tb = 4 * q + blk
                            vps = mm.tile([128, 256], F32, tag="mm",
                                          name=f"v{q}{blk}")
                            for ci in range(CCH):
                                nc.tensor.matmul(
                                    vps[:],
                                    xq_[:, ci, 128 * blk : 128 * (blk + 1)],
                                    W_sb[:, ci, 512:768],
                                    start=(ci == 0),
                                    stop=(ci == CCH - 1),
                                )
                            nc.vector.tensor_add(
                                vaug_h[:, tb, :, 0:64],
                                vps.rearrange("p (h c) -> p h c", c=64),
                                vbias_sb.rearrange("p (h c) -> p h c", c=64),
                            )
                        return emit
                    return [unit(blk) for blk in range(4)]

                # previous quarter's normalize fills DVE while this quarter's
                # qT/kT copies complete; this quarter's V units and the
                # previous quarter's proj units are spread into the ACT-bound
                # attention i-loop below as PE fillers (V first — it must
                # land before hp0's diagonal AVs in the drain)
                fillers_v = make_v_units()
                fillers_p = []
                if qtr > 0:
                    emit_norm()
                    fillers_p = make_proj_units(qtr - 1)

                # ---- attention for q_super j ----
                n_i = 4 * j + 4
                slots = max(1, 2 * n_i)
                n_fill = len(fillers_v) + len(fillers_p)
                slot = 0
                popped = 0
                for hp in range(PAIRS):
                    # flush hp0's normalize on the last quarter only (no next
                    # quarter to absorb it); mid-kernel it steals ACT/DVE from
                    # the exp pipeline
                    if qtr == NQS - 1:
                        emit_norm(bc_on_dve=True)
                    o_ps = {
                        h: opool.tile([65, QS], F32, tag="o", name=f"o{j}{hp}{h}")
                        for h in range(2)
                    }
                    ets = {}

                    def blk_qs0(t):
                        # f32r matmuls under 256 moving run at 4 cyc/row, so
                        # keep diag blocks >= 256 wide; the extra columns are
                        # zeroed by the mask before AV
                        return 0 if t < 0 else min(128 * t, QS - 256)

                    def emit_qk(i):
                        t = i - 4 * j
                        qs0 = blk_qs0(t)
                        sps = spool.tile([128, 2, QS], F32, tag="s",
                                         name=f"s{j}{hp}{i}")
                        for h in range(2):
                            pb = 64 * h
                            nc.tensor.matmul(
                                sps[:, h, qs0:],
                                kT[pb : pb + 64, hp, 128 * i : 128 * (i + 1)],
                                qT[pb : pb + 64, hp, QS * j + qs0 : QS * (j + 1)],
                                start=True,
                                stop=True,
                            )
                        et = et_pool.tile([128, 2, QS], F32R, tag="et",
                                          name=f"et{j}{hp}{i}")
                        nc.scalar.activation(
                            et[:, :, qs0:], sps[:, :, qs0:], Exp, scale=0.125,
                        )
                        if t >= 0:
                            # cover [qs0, end of triangle]; columns past the
                            # triangle are all-valid
                            mhi = 128 * t + 128
                            nc.vector.tensor_mul(
                                et[:, :, qs0:mhi],
                                et[:, :, qs0:mhi],
                                masks[:, t, qs0:mhi].unsqueeze(1)
                                .broadcast_to([128, 2, mhi - qs0]),
                            )
                        ets[i] = et

                    def emit_av(i):
                        t = i - 4 * j
                        qs0 = blk_qs0(t)
                        et = ets.pop(i)
                        for h in range(2):
                            hh = (2 * hp + h) * 65
                            nc.tensor.matmul(
                                o_ps[h][:, qs0:],
                                vaug[:, i, hh : hh + 65],
                                et[:, h, qs0:],
                                start=(i == 0),
                                stop=(i == n_i - 1),
                            )

                    LOOKAHEAD = 4
                    for i in range(n_i):
                        emit_qk(i)
                        if i >= LOOKAHEAD:
                            emit_av(i - LOOKAHEAD)
                        slot += 1
                        # spread fillers evenly across the quarter's two hp
                        # segments, skipping the first slots where PE is
                        # still dense with QK pipeline-fill
                        off = 0
                        while (fillers_v or fillers_p) and slot > off and \
                                (slot - off) * n_fill >= \
                                (popped + 1) * max(1, slots - off):
                            popped += 1
                            if fillers_v:
                                fillers_v.pop(0)()
                            else:
                                fillers_p.pop(0)()
                    if hp == 0:
                        # diagonal AVs (in the drain) consume this quarter's
                        # V — flush any V units the i-loop didn't absorb
                        while fillers_v:
                            fillers_v.pop(0)()
                    for i in range(max(0, n_i - LOOKAHEAD), n_i):
                        emit_av(i)

                    if hp == PAIRS - 1:
                        # hp1: defer the o drain into the flush (next quarter
                        # or tail) so it queues after the m-tile copybacks;
                        # the last quarter normalizes straight from PSUM
                        pending_norm.append((j, hp, o_ps))
                    else:
                        # hp0: drain o to SBUF now (DVE idle mid-attention)
                        # to free PSUM for hp1
                        osb = {}
                        for h in range(2):
                            osb[h] = work.tile([65, QS], F32, tag="osb",
                                               bufs=4, name=f"osb{j}{hp}{h}")
                            nc.vector.tensor_copy(osb[h][:], o_ps[h][:])
                        pending_norm.append((j, hp, osb))

                # any proj units not absorbed by the i-loop
                for u in fillers_p:
                    u()

            emit_norm()
            emit_proj(NQS - 1, tail=True)

    nc.compile()
    return nc


def _host_prep(x, W_qkv, b_qkv, W_proj, b_proj):
    """Build per-core input maps."""
    x = np.asarray(x, dtype=np.float32)
    W_qkv = np.asarray(W_qkv, dtype=np.float32)
    b_qkv = np.asarray(b_qkv, dtype=np.float32)
    W_proj = np.asarray(W_proj, dtype=np.float32)

    ones64D = np.ones((1, 64), dtype=np.float32)

    xTs = [np.ascontiguousarray(x[b].T) for b in range(B)]

    in_maps = []
    for c in range(NCORES):
        b, g = divmod(c, GROUPS)
        cols = slice(256 * g, 256 * (g + 1))
        Wslice = np.ascontiguousarray(
            np.concatenate(
                [W_qkv[:, cols], W_qkv[:, 1024:2048][:, cols],
                 W_qkv[:, 2048:3072][:, cols]],
                axis=1,
            )
        )
        bq = b_qkv[cols.start : cols.stop]
        bk = b_qkv[1024 + cols.start : 1024 + cols.stop]
        bv = b_qkv[2048 + cols.start : 2048 + cols.stop]
        biasqk = np.ascontiguousarray(
            np.stack([bq[:128], bq[128:], bk[:128], bk[128:]], axis=1)
        )
        vbias = np.ascontiguousarray(np.broadcast_to(bv, (128, 256)))
        Wp_slice = np.ascontiguousarray(W_proj[cols])
        in_maps.append(
            {
                "xT": xTs[b],
                "W": Wslice,
                "Wp": Wp_slice,
                "biasqk": biasqk,
                "vbias": vbias,
                "ones64D": ones64D,
            }
        )
    return in_maps


def _make_runner(nc):
    """Build the PJRT executable once (mirrors bass2jax.run_bass_via_pjrt)
    so repeated kernel() calls skip re-tracing/compile-cache lookups."""
    import jax
    from jax.sharding import Mesh, PartitionSpec
    from jax.experimental.shard_map import shard_map

    from concourse.bass2jax import (
        _bass_exec_p,
        install_neuronx_cc_hook,
        partition_id_tensor,
    )

    install_neuronx_cc_hook()
    partition_name = (
        nc.partition_id_tensor.name if nc.partition_id_tensor else None
    )
    in_names, out_names, out_avals, zero_outs = [], [], [], []
    for alloc in nc.m.functions[0].allocations:
        if not isinstance(alloc, mybir.MemoryLocationSet):
            continue
        name = alloc.memorylocations[0].name
        if alloc.kind == "ExternalInput":
            if name != partition_name:
                in_names.append(name)
        elif alloc.kind == "ExternalOutput":
            out_names.append(name)
            shape = tuple(alloc.tensor_shape)
            dtype = mybir.dt.np(alloc.dtype)
            out_avals.append(jax.core.ShapedArray(shape, dtype))
            zero_outs.append(np.zeros(shape, dtype))
    n_params = len(in_names)
    all_in_names = in_names + out_names
    if partition_name is not None:
        all_in_names = all_in_names + [partition_name]

    def _body(*args):
        operands = list(args)
        if partition_name is not None:
            operands.append(partition_id_tensor())
        return tuple(
            _bass_exec_p.bind(
                *operands,
                out_avals=tuple(out_avals),
                in_names=tuple(all_in_names),
                out_names=tuple(out_names),
                lowering_input_output_aliases=(),
                sim_require_finite=True,
                sim_require_nnan=True,
                nc=nc,
            )
        )

    devices = jax.devices()[:NCORES]
    mesh = Mesh(np.asarray(devices), ("core",))
    in_specs = (PartitionSpec("core"),) * (n_params + len(out_names))
    out_specs = (PartitionSpec("core"),) * len(out_names)
    fn = jax.jit(
        shard_map(_body, mesh=mesh, in_specs=in_specs,
                  out_specs=out_specs, check_rep=False),
        keep_unused=True,
    )
    concat_zeros = [
        np.zeros((NCORES * z.shape[0], *z.shape[1:]), z.dtype)
        for z in zero_outs
    ]

    def run(in_maps):
        concat_in = [
            np.concatenate([np.asarray(m[name]) for m in in_maps], axis=0)
            for name in in_names
        ]
        out_arrs = fn(*concat_in, *concat_zeros)
        return [
            {
                name: np.asarray(out_arrs[i]).reshape(
                    NCORES, *out_avals[i].shape
                )[c]
                for i, name in enumerate(out_names)
            }
            for c in range(NCORES)
        ]

    return run


def kernel(x, W_qkv, b_qkv, W_proj, b_proj):
    if "nc" not in _CACHE:
        _CACHE["nc"] = _build()
        try:
            _CACHE["run"] = _make_runner(_CACHE["nc"])
        except Exception:
            _CACHE["run"] = None
    in_maps = _host_prep(x, W_qkv, b_qkv, W_proj, b_proj)
    results = None
    if _CACHE["run"] is not None:
        try:
            results = _CACHE["run"](in_maps)
        except Exception:
            results = None
    if results is None:
        # fallback: the stock path
        results = run_bass_kernel_spmd(
            _CACHE["nc"], in_maps, core_ids=list(range(NCORES))
        ).results
    out = np.zeros((B, N, D), dtype=np.float32)
    bp = np.asarray(b_proj, dtype=np.float32)
    for b in range(B):
        acc = results[4 * b]["y"].astype(np.float32).copy()
        for g in range(1, GROUPS):
            acc += results[4 * b + g]["y"]
        out[b] = acc + bp
    return out



# revision 9
# speedup vs baseline: 1.1083x; 1.1083x over previous
"""Causal self-attention (b=2, n=2048, d=1024, 16 heads) on 8 NeuronCores.

Sharding: core c handles batch b = c // 4 and head group g4 = c % 4
(heads 4*g4..4*g4+3).  qkv weights column-sharded, proj weights row-sharded
(Megatron); each core emits a partial [2048, 1024] proj output and the
host sums the 4 partials per batch (b_proj added host-side).

Engine plan (per core, cost-model driven):
  q/k projection : fp8e4m3 DoubleRow matmuls (x fp8, W*64 fp8); PSUM
                   drained to fp8 q/k tiles with the bias folded in.
  S = K^T Q      : fp8 DoubleRow, Kp=32 (hd split 2x32), heads paired in
                   [64,2,N] tiles at partition bases 0/32; exact causal
                   triangle at 128-col granularity; exp folds the 1/8
                   softmax scale and the 1/64^2 fp8 weight scale.
  exp            : ACT, PSUM f32 -> SBUF bf16 (ACT is the bottleneck
                   engine; everything else is scheduled around it).
  causal mask    : gpsimd affine_select on the diagonal et blocks.
  A @ V          : flipped: et [128k,128q] stationary (bf16), [V|1]
                   [128k,65] moving -> token-major o [128q,65] with the
                   softmax denominator in column 64.  Four 65-col
                   accumulation regions share one PSUM bank: only the
                   first matmul into the bank uses start=True (bank-wide
                   pending-zero), later regions' first matmul overwrites
                   via the pending-zero bits.
  normalize      : DVE reciprocal of col 64 + per-(token,head) multiply,
                   output bf16 token-major.
  transpose      : PE is_transpose (bf16, identity rhs) back to
                   feature-major onT for the output projection.
  projection     : bf16 onT stationary x bf16 Wp moving; f32 y out.
"""
import sys

sys.path.insert(0, "/opt/trn_rl_repo")

import numpy as np

import concourse.bass as bass  # noqa: F401
import concourse.mybir as mybir
import concourse.tile as tile
from concourse import bacc
from concourse.bass_utils import run_bass_kernel_spmd

F32 = mybir.dt.float32
F32R = mybir.dt.float32r
BF16 = mybir.dt.bfloat16
FP8 = mybir.dt.float8e4
Exp = mybir.ActivationFunctionType.Exp
DR = mybir.MatmulPerfMode.DoubleRow

B = 2
N = 2048
D = 1024
NH = 16
HD = 64
NCORES = 8
GROUPS = 4                # head groups (cores per batch)
HPC = NH // GROUPS        # heads per core = 4
QS = 512                  # q_super width
NQS = N // QS             # 4
NB = N // 128             # 16 token blocks
CCH = D // 128            # 8 contraction chunks
KP = CCH // 2             # 4 DoubleRow contraction pairs
EXP_SCALE = 0.125

_CACHE = {}


def _build():
    nc = bacc.Bacc("TRN2", target_bir_lowering=False, debug=False,
                   num_devices=NCORES)
    xv = nc.dram_tensor("xv", [D, N], BF16, kind="ExternalInput").ap()
    Wqk = nc.dram_tensor("Wqk", [128, CCH * 512], BF16,
                         kind="ExternalInput").ap()
    Wv = nc.dram_tensor("Wv", [128, CCH * 256], BF16, kind="ExternalInput").ap()
    Wp = nc.dram_tensor("Wp", [128, 2 * D], BF16, kind="ExternalInput").ap()
    bqk = nc.dram_tensor("bqk", [128, 4], F32, kind="ExternalInput").ap()
    vbias = nc.dram_tensor("vbias", [128, 256], F32, kind="ExternalInput").ap()
    identD = nc.dram_tensor("identD", [128, 128], BF16, kind="ExternalInput").ap()
    y = nc.dram_tensor("y", [N, D], F32, kind="ExternalOutput").ap()

    with tile.TileContext(nc) as tc:
        with (
            tc.tile_pool(name="persist", bufs=1) as pp,
            tc.tile_pool(name="xv_pool", bufs=2) as xv_pool,
            tc.tile_pool(name="et_pool", bufs=6) as et_pool,
            tc.tile_pool(name="onorm_pool", bufs=2) as onorm_pool,
            tc.tile_pool(name="work", bufs=4) as work,
            tc.tile_pool(name="ysb_pool", bufs=4) as ysb_pool,
            tc.tile_pool(name="mm", bufs=2, space="PSUM") as mm,
            tc.tile_pool(name="spool", bufs=2, space="PSUM") as spool,
            tc.tile_pool(name="opool", bufs=2, space="PSUM") as opool,
        ):
            # ---- persistent tiles ----
            Wqk_sb = pp.tile([128, CCH, 4, 128], BF16)     # (chunk, tile, m)
            Wv_sb = pp.tile([128, CCH, 256], BF16)
            Wp_sb = pp.tile([128, 2, D], BF16)
            bqk_sb = pp.tile([128, 4], F32)                # per (qk,i) tile
            vb_sb = pp.tile([128, 256], F32)
            ident = pp.tile([128, 128], BF16)
            # q/k bf16, heads paired: qkT[g][64*l + hd, qk, n] for heads 2g+l
            qkT = {g: pp.tile([128, 2, N], BF16, name=f"qkT_{g}")
                   for g in range(2)}
            vaug = pp.tile([128, NB, HPC, 65], BF16)       # [V | 1] token-major
            onT = pp.tile([128, 2, N], BF16)               # feature-major o

            Wqk_r = Wqk.rearrange("p (c t m) -> p c t m", c=CCH, t=4)
            xv_r = xv.rearrange("(c p) n -> p c n", p=128)
            Wv_r = Wv.rearrange("p (c f) -> p c f", c=CCH)
            Wp_r = Wp.rearrange("p (c f) -> p c f", c=2)
            y_r = y.rearrange("(t p) f -> t p f", p=128)

            def fetch_x(q):
                t0, t1 = QS * q, QS * (q + 1)
                xvq = xv_pool.tile([128, CCH, QS], BF16, tag="xv",
                                   name=f"xv_{q}")
                nc.sync.dma_start(xvq[:], xv_r[:, :, t0:t1])
                return xvq

            # ---------- per-quarter state ----------
            pending = []          # closures from quarter j-1 (norm chain+proj)
            next_x = None

            for j in range(NQS):
                ts, te = QS * j, QS * (j + 1)
                n_i = 4 * j + 4

                # ---- input DMAs, ordered by first consumption ----
                if j == 0:
                    nc.sync.dma_start(Wqk_sb[:], Wqk_r)
                    nc.sync.dma_start(bqk_sb[:], bqk)
                    xvq = fetch_x(0)
                    nc.sync.dma_start(Wv_sb[:], Wv_r)
                    nc.sync.dma_start(vb_sb[:], vbias)
                    nc.sync.dma_start(ident[:], identD)
                    nc.sync.dma_start(Wp_sb[:], Wp_r)
                    # ones columns of [V|1] via DVE (0*x + 1)
                    nc.vector.tensor_scalar(
                        out=vaug[:, :, :, 64],
                        in0=ident[:, 0:64].rearrange("p (a b) -> p a b", b=HPC),
                        scalar1=0.0,
                        scalar2=1.0,
                        op0=mybir.AluOpType.mult,
                        op1=mybir.AluOpType.add,
                    )
                    next_x = fetch_x(1)
                else:
                    xvq = next_x
                    if j + 1 < NQS:
                        next_x = fetch_x(j + 1)

                # ---- q/k projection: 4 (qk, g) tiles, bf16 ----
                # tile t = 2*qk + g: 128 features = heads {2g, 2g+1}
                for t in range(4):
                    ps = mm.tile([128, QS], F32, tag="mm", name=f"qk{j}{t}")
                    for ci in range(CCH):
                        nc.tensor.matmul(
                            ps[:],
                            Wqk_sb[:, ci, t, :],
                            xvq[:, ci, :],
                            start=(ci == 0),
                            stop=(ci == CCH - 1),
                        )
                    qk, g = divmod(t, 2)
                    nc.vector.tensor_scalar_add(
                        qkT[g][:, qk, ts:te],
                        ps[:],
                        bqk_sb[:, t : t + 1],
                    )

                # ---- previous quarter's g1 norm chain: run now so the
                #      o-pool slots free before this quarter's first AV ----
                for u_ in pending[:2]:
                    u_()
                queue = pending[2:]
                pending = []

                # ---- V units (token-major, consumed by this quarter's
                #      diagonal AVs) ----
                def make_v_units(q=j, xv_=xvq):
                    def unit(blk):
                        def emit():
                            tb = 4 * q + blk
                            vps = mm.tile([128, 256], F32, tag="mm",
                                          name=f"v{q}{blk}")
                            for ci in range(CCH):
                                nc.tensor.matmul(
                                    vps[:],
                                    xv_[:, ci, 128 * blk : 128 * (blk + 1)],
                                    Wv_sb[:, ci, :],
                                    start=(ci == 0),
                                    stop=(ci == CCH - 1),
                                )
                            nc.vector.tensor_add(
                                vaug[:, tb, :, 0:64],
                                vps.rearrange("p (h c) -> p h c", c=64),
                                vb_sb.rearrange("p (h c) -> p h c", c=64),
                            )
                        return emit
                    return [unit(blk) for blk in range(4)]

                v_units = make_v_units()

                # ---- attention for q_super j, per head pair g ----
                for g in range(2):
                    o_ps = {
                        half: opool.tile([128, 2, 2, 65], F32, tag="o",
                                         name=f"o{j}{g}{half}")
                        for half in range(2)
                    }
                    first_touch = {half: True for half in range(2)}
                    ets = {}

                    def emit_qk(i, g=g, j=j, ets=ets):
                        t = i - 4 * j
                        qs0 = 128 * t if t >= 0 else 0
                        sps = spool.tile([128, 2, QS], F32, tag="s",
                                         name=f"s{j}{g}{i}")
                        for l in range(2):
                            nc.tensor.matmul(
                                sps[:, l, qs0:],
                                qkT[g][64 * l : 64 * (l + 1), 1,
                                       128 * i : 128 * (i + 1)],
                                qkT[g][64 * l : 64 * (l + 1), 0,
                                       QS * j + qs0 : QS * (j + 1)],
                                start=True,
                                stop=True,
                            )
                        et = et_pool.tile([128, 2, QS], BF16, tag="et",
                                          name=f"et{j}{g}{i}")
                        nc.scalar.activation(
                            et[:, :, qs0:], sps[:, :, qs0:], Exp,
                            scale=EXP_SCALE,
                        )
                        if t >= 0:
                            # mask the diagonal 128-block on gpsimd
                            nc.gpsimd.affine_select(
                                out=et[:, :, qs0 : qs0 + 128],
                                in_=et[:, :, qs0 : qs0 + 128],
                                compare_op=mybir.AluOpType.is_ge,
                                fill=0.0,
                                base=0,
                                pattern=[[0, 2], [1, 128]],
                                channel_multiplier=-1,
                            )
                        ets[i] = et

                    def emit_av(i, g=g, j=j, ets=ets, o_ps=o_ps,
                                first_touch=first_touch):
                        t = i - 4 * j
                        et = ets.pop(i)
                        for u in range(max(0, t), 4):
                            half, u2 = divmod(u, 2)
                            for l in range(2):
                                st = first_touch[half]
                                first_touch[half] = False
                                nc.tensor.matmul(
                                    o_ps[half][:, l, u2, :],
                                    et[:, l, 128 * u : 128 * (u + 1)],
                                    vaug[:, i, 2 * g + l, :],
                                    start=st,
                                    stop=(i == 4 * j + u),
                                    skip_group_check=True,
                                )

                    LOOKAHEAD = 3
                    for i in range(n_i):
                        t = i - 4 * j
                        if g == 0 and t >= 0:
                            # diagonal AV(i) consumes vaug[4j+t]: flush V
                            while len(v_units) > 3 - t:
                                v_units.pop(0)()
                        emit_qk(i)
                        if i >= LOOKAHEAD:
                            emit_av(i - LOOKAHEAD)
                        # one filler per slot: V early in g0, then the queue
                        if g == 0 and i < 4 and v_units:
                            v_units.pop(0)()
                        elif queue:
                            queue.pop(0)()
                    for i in range(max(0, n_i - LOOKAHEAD), n_i):
                        emit_av(i)

                    # ---- deferred normalize + transpose chain for (j, g) ----
                    def make_norm(j=j, g=g, o_ps=o_ps):
                        onorm = {}

                        def norm():
                            on = onorm_pool.tile([128, 4, 2, 64], BF16,
                                                 tag="onorm", name=f"on{j}{g}")
                            onorm[0] = on
                            for half in range(2):
                                rc = work.tile([128, 2, 2], F32, tag="recip",
                                               name=f"rc{j}{g}{half}")
                                nc.vector.reciprocal(
                                    rc[:], o_ps[half][:, :, :, 64])
                                nc.vector.tensor_mul(
                                    on[:, 2 * half : 2 * half + 2, :, :],
                                    o_ps[half][:, :, :, 0:64]
                                        .rearrange("p s u c -> p u s c"),
                                    rc.rearrange("p s u -> p u s")
                                        .unsqueeze(3)
                                        .broadcast_to([128, 2, 2, 64]),
                                )

                        def transp():
                            trp = mm.tile([128, 4, 128], BF16, tag="mm",
                                          name=f"tr{j}{g}")
                            on = onorm[0]
                            for u in range(4):
                                nc.tensor.matmul(
                                    trp[:, u, :],
                                    on[:, u, :, :],
                                    ident[:],
                                    start=True,
                                    stop=True,
                                    is_transpose=True,
                                )
                            nc.vector.tensor_copy(
                                onT[:, g, QS * j : QS * (j + 1)],
                                trp.rearrange("p u q -> p (u q)"),
                            )

                        return [norm, transp]

                    if g == 0:
                        # run in g1's segment so the o slots recycle promptly
                        queue = make_norm() + queue
                    else:
                        pending.extend(make_norm())

                # ---- output projection units for quarter j (deferred) ----
                def make_proj(jj=j):
                    tail = jj == NQS - 1
                    ysbs = {}

                    def unit(blk, nh):
                        def emit():
                            tb = 4 * jj + blk
                            yps = mm.tile([128, QS], F32, tag="mm",
                                          name=f"y{tb}{nh}")
                            for c in range(2):
                                nc.tensor.matmul(
                                    yps[:],
                                    onT[:, c, 128 * tb : 128 * (tb + 1)],
                                    Wp_sb[:, c, QS * nh : QS * (nh + 1)],
                                    start=(c == 0),
                                    stop=(c == 1),
                                )
                            if nh == 0:
                                ysbs[tb] = ysb_pool.tile(
                                    [128, 2, QS], F32, tag="ysb", bufs=4,
                                    name=f"ysb{tb}")
                            ysb = ysbs[tb]
                            if tail and (blk + nh) % 2 == 1:
                                nc.scalar.copy(ysb[:, nh, :], yps[:])
                            else:
                                nc.vector.tensor_copy(ysb[:, nh, :], yps[:])
                            if nh == 1:
                                nc.sync.dma_start(
                                    y_r[tb], ysb.rearrange("p a b -> p (a b)"))
                        return emit
                    return unit

                # flush any unpopped fillers before the next quarter
                while queue:
                    queue.pop(0)()

                pu = make_proj()
                pending.extend(pu(blk, nh) for blk in range(4)
                               for nh in range(2))

            # ---- tail: last quarter's norm chain + projection ----
            for f in pending:
                f()

    nc.compile()
    return nc


def _host_prep(x, W_qkv, b_qkv, W_proj, b_proj):
    """Build per-core input maps."""
    import ml_dtypes
    f8 = ml_dtypes.float8_e4m3
    bf = ml_dtypes.bfloat16

    x = np.asarray(x, dtype=np.float32)
    W_qkv = np.asarray(W_qkv, dtype=np.float32)
    b_qkv = np.asarray(b_qkv, dtype=np.float32)
    W_proj = np.asarray(W_proj, dtype=np.float32)

    xvs = [np.ascontiguousarray(x[b].T).astype(bf) for b in range(B)]
    ident = np.eye(128, dtype=np.float32).astype(bf)

    in_maps = []
    for c in range(NCORES):
        b, g4 = divmod(c, GROUPS)
        col0 = 256 * g4

        # Wqk[p, c, t, m]: tile t = 2*qk + g holds heads {2g, 2g+1}
        Wqk = np.zeros((128, CCH, 4, 128), dtype=np.float32)
        bqk = np.zeros((128, 4), dtype=np.float32)
        for t in range(4):
            qk, g = divmod(t, 2)
            c0 = 1024 * qk + col0 + 128 * g
            Wqk[:, :, t, :] = (
                W_qkv[:, c0 : c0 + 128].reshape(CCH, 128, 128)
                .transpose(1, 0, 2)
            )
            bqk[:, t] = b_qkv[c0 : c0 + 128]
        Wqk = np.ascontiguousarray(Wqk.reshape(128, CCH * 512)).astype(bf)

        Wv = np.ascontiguousarray(
            W_qkv[:, 2048 + col0 : 2048 + col0 + 256]
            .reshape(CCH, 128, 256).transpose(1, 0, 2)
            .reshape(128, CCH * 256)
        ).astype(bf)
        bv = b_qkv[2048 + col0 : 2048 + col0 + 256]
        vbias = np.ascontiguousarray(
            np.broadcast_to(bv, (128, 256))).astype(np.float32)
        Wp = np.ascontiguousarray(
            W_proj[col0 : col0 + 256].reshape(2, 128, D).transpose(1, 0, 2)
            .reshape(128, 2 * D)
        ).astype(bf)
        in_maps.append(
            {
                "xv": xvs[b],
                "Wqk": Wqk,
                "Wv": Wv,
                "Wp": Wp,
                "bqk": bqk,
                "vbias": vbias,
                "identD": ident,
            }
        )
    return in_maps


def _make_runner(nc):
    """Build the PJRT executable once (mirrors bass2jax.run_bass_via_pjrt)
    so repeated kernel() calls skip re-tracing/compile-cache lookups."""
    import jax
    from jax.sharding import Mesh, PartitionSpec
    from jax.experimental.shard_map import shard_map

    from concourse.bass2jax import (
        _bass_exec_p,
        install_neuronx_cc_hook,
        partition_id_tensor,
    )

    install_neuronx_cc_hook()
    partition_name = (
        nc.partition_id_tensor.name if nc.partition_id_tensor else None
    )
    in_names, out_names, out_avals, zero_outs = [], [], [], []
    for alloc in nc.m.functions[0].allocations:
        if not isinstance(alloc, mybir.MemoryLocationSet):
            continue
        name = alloc.memorylocations[0].name
        if alloc.kind == "ExternalInput":
            if name != partition_name:
                in_names.append(name)
        elif alloc.kind == "ExternalOutput":
            out_names.append(name)
            shape = tuple(alloc.tensor_shape)
            dtype = mybir.dt.np(alloc.dtype)
            out_avals.append(jax.core.ShapedArray(shape, dtype))
            zero_outs.append(np.zeros(shape, dtype))
    n_params = len(in_names)
    all_in_names = in_names + out_names
    if partition_name is not None:
        all_in_names = all_in_names + [partition_name]

    def _body(*args):
        operands = list(args)
        if partition_name is not None:
            operands.append(partition_id_tensor())
        return tuple(
            _bass_exec_p.bind(
                *operands,
                out_avals=tuple(out_avals),
                in_names=tuple(all_in_names),
                out_names=tuple(out_names),
                lowering_input_output_aliases=(),
                sim_require_finite=True,
                sim_require_nnan=True,
                nc=nc,
            )
        )

    devices = jax.devices()[:NCORES]
    mesh = Mesh(np.asarray(devices), ("core",))
    in_specs = (PartitionSpec("core"),) * (n_params + len(out_names))
    out_specs = (PartitionSpec("core"),) * len(out_names)
    fn = jax.jit(
        shard_map(_body, mesh=mesh, in_specs=in_specs,
                  out_specs=out_specs, check_rep=False),
        keep_unused=True,
    )
    concat_zeros = [
        np.zeros((NCORES * z.shape[0], *z.shape[1:]), z.dtype)
        for z in zero_outs
    ]

    def run(in_maps):
        concat_in = [
            np.concatenate([np.asarray(m[name]) for m in in_maps], axis=0)
            for name in in_names
        ]
        out_arrs = fn(*concat_in, *concat_zeros)
        return [
            {
                name: np.asarray(out_arrs[i]).reshape(
                    NCORES, *out_avals[i].shape
                )[c]
                for i, name in enumerate(out_names)
            }
            for c in range(NCORES)
        ]

    return run


def kernel(x, W_qkv, b_qkv, W_proj, b_proj):
    if "nc" not in _CACHE:
        _CACHE["nc"] = _build()
        try:
            _CACHE["run"] = _make_runner(_CACHE["nc"])
        except Exception:
            _CACHE["run"] = None
    in_maps = _host_prep(x, W_qkv, b_qkv, W_proj, b_proj)
    results = None
    if _CACHE["run"] is not None:
        try:
            results = _CACHE["run"](in_maps)
        except Exception:
            results = None
    if results is None:
        results = run_bass_kernel_spmd(
            _CACHE["nc"], in_maps, core_ids=list(range(NCORES))
        ).results
    out = np.zeros((B, N, D), dtype=np.float32)
    bp = np.asarray(b_proj, dtype=np.float32)
    for b in range(B):
        acc = results[4 * b]["y"].astype(np.float32).copy()
        for g in range(1, GROUPS):
            acc += results[4 * b + g]["y"]
        out[b] = acc + bp
    return out


# revision 17
# speedup vs baseline: 1.2302x; 1.1100x over previous
"""Causal self-attention (b=2, n=2048, d=1024, 16 heads) on 8 NeuronCores.

Sharding: core c handles batch b = c // 4 and head group g4 = c % 4
(heads 4*g4..4*g4+3).  qkv weights column-sharded, proj weights row-sharded
(Megatron); each core emits a partial [2048, 1024] proj output and the
host sums the 4 partials per batch (b_proj added host-side).

Engine plan (per core, cost-model driven):
  q/k/v projection : fp8e4m3 DoubleRow with residual compensation:
                     x -> f8(4x) + f8(residual), W -> f8(256W) +
                     f8(residual); acc = x8@W8 + xr8@W8 + x8@Wr8 in one
                     PSUM group (all terms share the 1024x scale), giving
                     ~0.2% error at 1.5x the fp8 matmul cost.
  S = K^T Q        : bf16 (q/k fp8 would put ~1.5%/quantization of
                     correlated tilt into softmax - too close to the 2e-2
                     gate), heads at partition bases 0/64, exact causal
                     triangle at 128-col granularity; exp folds the 1/8
                     softmax scale and the 2^-20 projection scale.
  exp              : ACT, PSUM f32 -> SBUF bf16.
  causal mask      : gpsimd affine_select on the diagonal et blocks.
  A @ V            : flipped: et [128k,128q] stationary (bf16), [V|1]
                     [128k,65] moving -> token-major o [128q,65] with the
                     softmax denominator in column 64.  Four 65-col
                     accumulation regions share one PSUM bank: only the
                     first matmul into the bank uses start=True (bank-wide
                     pending-zero), later regions' first matmul overwrites
                     via the pending-zero bits.
  normalize        : DVE reciprocal of col 64 + per-(token,head) multiply,
                     output bf16 token-major.
  transpose        : PE is_transpose (bf16, identity rhs) back to
                     feature-major onT for the output projection.
  projection       : bf16 onT stationary x bf16 Wp moving; f32 y out.

Scheduling: PE work (~92us) exceeds ACT exp (~77us), so QK->exp supply is
first-class; V/qkproj(j+1)/norm units drain at a fixed slot rate while
deferrable proj units pop only when the quarter's emitted exp time exceeds
its emitted PE time (leftovers carry to the next quarter / tail).
"""
import sys

sys.path.insert(0, "/opt/trn_rl_repo")

import numpy as np

import concourse.bass as bass  # noqa: F401
import concourse.mybir as mybir
import concourse.tile as tile
from concourse import bacc
from concourse.bass_utils import run_bass_kernel_spmd

F32 = mybir.dt.float32
F32R = mybir.dt.float32r
BF16 = mybir.dt.bfloat16
FP8 = mybir.dt.float8e4
Exp = mybir.ActivationFunctionType.Exp
DR = mybir.MatmulPerfMode.DoubleRow

B = 2
N = 2048
D = 1024
NH = 16
HD = 64
NCORES = 8
GROUPS = 4                # head groups (cores per batch)
HPC = NH // GROUPS        # heads per core = 4
QS = 512                  # q_super width
NQS = N // QS             # 4
NB = N // 128             # 16 token blocks
CCH = D // 128            # 8 contraction chunks
KP = CCH // 2             # 4 DoubleRow contraction pairs
XS = 4.0                  # fp8 scale on x
WS = 256.0                # fp8 scale on W_qkv
EXP_SCALE = 0.125 / (XS * WS) ** 2     # q,k both carry the 1024x scale
VSCALE = 1.0 / (XS * WS)               # V drain rescale

_CACHE = {}


def _build():
    nc = bacc.Bacc("TRN2", target_bir_lowering=False, debug=False,
                   num_devices=NCORES)
    x8d = nc.dram_tensor("x8", [D, N], FP8, kind="ExternalInput").ap()
    xr8d = nc.dram_tensor("xr8", [D, N], FP8, kind="ExternalInput").ap()
    W8d = nc.dram_tensor("W8", [128, KP * 2 * 4 * 128], FP8,
                         kind="ExternalInput").ap()
    Wr8d = nc.dram_tensor("Wr8", [128, KP * 2 * 4 * 128], FP8,
                          kind="ExternalInput").ap()
    Wv8d = nc.dram_tensor("Wv8", [128, KP * 2 * 256], FP8,
                          kind="ExternalInput").ap()
    Wvr8d = nc.dram_tensor("Wvr8", [128, KP * 2 * 256], FP8,
                           kind="ExternalInput").ap()
    Wp = nc.dram_tensor("Wp", [128, 2 * D], BF16, kind="ExternalInput").ap()
    bqk = nc.dram_tensor("bqk", [128, 4], F32, kind="ExternalInput").ap()
    vbias = nc.dram_tensor("vbias", [128, 256], F32, kind="ExternalInput").ap()
    identD = nc.dram_tensor("identD", [128, 128], BF16, kind="ExternalInput").ap()
    y = nc.dram_tensor("y", [N, D], F32, kind="ExternalOutput").ap()

    with tile.TileContext(nc) as tc:
        with (
            tc.tile_pool(name="persist", bufs=1) as pp,
            tc.tile_pool(name="x8_pool", bufs=2) as x8_pool,
            tc.tile_pool(name="xr8_pool", bufs=2) as xr8_pool,
            tc.tile_pool(name="et_pool", bufs=6) as et_pool,
            tc.tile_pool(name="onorm_pool", bufs=2) as onorm_pool,
            tc.tile_pool(name="work", bufs=4) as work,
            tc.tile_pool(name="ysb_pool", bufs=4) as ysb_pool,
            tc.tile_pool(name="mm", bufs=2, space="PSUM") as mm,
            tc.tile_pool(name="spool", bufs=2, space="PSUM") as spool,
            tc.tile_pool(name="opool", bufs=2, space="PSUM") as opool,
        ):
            # ---- persistent tiles ----
            W8_sb = pp.tile([128, KP, 2, 4, 128], FP8)    # (kp, two, tile, m)
            Wr8_sb = pp.tile([128, KP, 2, 4, 128], FP8)
            Wv8_sb = pp.tile([128, KP, 2, 256], FP8)
            Wvr8_sb = pp.tile([128, KP, 2, 256], FP8)
            Wp_sb = pp.tile([128, 2, D], BF16)
            bqk_sb = pp.tile([128, 4], F32)               # per (qk,g) tile
            vb_sb = pp.tile([128, 256], F32)
            ident = pp.tile([128, 128], BF16)
            # q/k bf16 (1024x scaled), heads paired at bases 0/64
            qkT = {g: pp.tile([128, 2, N], BF16, name=f"qkT_{g}")
                   for g in range(2)}
            vaug = pp.tile([128, NB, HPC, 65], BF16)      # [V | 1] token-major
            onT = pp.tile([128, 2, N], BF16)              # feature-major o

            W8_r = W8d.rearrange("p (kp two t m) -> p kp two t m",
                                 kp=KP, two=2, t=4)
            Wr8_r = Wr8d.rearrange("p (kp two t m) -> p kp two t m",
                                   kp=KP, two=2, t=4)
            Wv8_r = Wv8d.rearrange("p (kp two f) -> p kp two f", kp=KP, two=2)
            Wvr8_r = Wvr8d.rearrange("p (kp two f) -> p kp two f",
                                     kp=KP, two=2)
            x8_r = x8d.rearrange("(kp two p) n -> p kp two n", p=128, two=2)
            xr8_r = xr8d.rearrange("(kp two p) n -> p kp two n", p=128, two=2)
            Wp_r = Wp.rearrange("p (c f) -> p c f", c=2)
            y_r = y.rearrange("(t p) f -> t p f", p=128)

            def fetch_x(q):
                t0, t1 = QS * q, QS * (q + 1)
                x8q = x8_pool.tile([128, KP, 2, QS], FP8, tag="x8",
                                   name=f"x8_{q}")
                nc.sync.dma_start(x8q[:], x8_r[:, :, :, t0:t1])
                xr8q = xr8_pool.tile([128, KP, 2, QS], FP8, tag="xr8",
                                     name=f"xr8_{q}")
                nc.sync.dma_start(xr8q[:], xr8_r[:, :, :, t0:t1])
                return x8q, xr8q

            # ---------- per-quarter state ----------
            pending = []       # from quarter j-1: [norm_g1, transp_g1] + proj
            carry = []         # deferrable units carried across quarters
            next_x = None
            # pacing state (reset per quarter): ns of exp emitted vs ns of
            # PE emitted
            bal = {"act": 0.0, "pe": 0.0}

            def pe_note(ns):
                bal["pe"] += ns

            for j in range(NQS):
                ts, te = QS * j, QS * (j + 1)
                n_i = 4 * j + 4
                bal["act"] = 0.0
                bal["pe"] = 0.0

                # ---- input DMAs, ordered by first consumption ----
                if j == 0:
                    # quarter-0 critical path: W8/Wr8 first, then per-kp
                    # x8/xr8 chunks so qkproj tracks chunk arrivals
                    nc.sync.dma_start(W8_sb[:], W8_r)
                    nc.sync.dma_start(Wr8_sb[:], Wr8_r)
                    nc.sync.dma_start(bqk_sb[:], bqk)
                    x8q = x8_pool.tile([128, KP, 2, QS], FP8, tag="x8",
                                       name="x8_0")
                    xr8q = xr8_pool.tile([128, KP, 2, QS], FP8, tag="xr8",
                                         name="xr8_0")
                    for kp in range(KP):
                        nc.sync.dma_start(x8q[:, kp, :, :],
                                          x8_r[:, kp, :, 0:QS])
                        nc.sync.dma_start(xr8q[:, kp, :, :],
                                          xr8_r[:, kp, :, 0:QS])
                    nc.sync.dma_start(ident[:], identD)
                    nc.sync.dma_start(Wv8_sb[:], Wv8_r)
                    nc.sync.dma_start(Wvr8_sb[:], Wvr8_r)
                    nc.sync.dma_start(vb_sb[:], vbias)
                    nc.sync.dma_start(Wp_sb[:], Wp_r)
                    # ones columns of [V|1] via DVE (0*x + 1)
                    nc.vector.tensor_scalar(
                        out=vaug[:, :, :, 64],
                        in0=ident[:, 0:64].rearrange("p (a b) -> p a b", b=HPC),
                        scalar1=0.0,
                        scalar2=1.0,
                        op0=mybir.AluOpType.mult,
                        op1=mybir.AluOpType.add,
                    )
                    next_x = fetch_x(1)
                else:
                    x8q, xr8q = next_x
                    if j + 1 < NQS:
                        next_x = fetch_x(j + 1)

                # ---- q/k projection: tile t = 2*qk + g = 128 features of
                # heads {2g, 2g+1}; three DoubleRow chains share one PSUM
                # accumulation (all at the 1024x scale).
                def make_qkproj_units(jq, xs_, xrs_):
                    tsq, teq = QS * jq, QS * (jq + 1)
                    pss = {}
                    chains = [(W8_sb, xs_), (W8_sb, xrs_), (Wr8_sb, xs_)]

                    def unit(t, ch):
                        def emit():
                            if ch == 0:
                                pss[t] = mm.tile([128, QS], F32, tag="mm",
                                                 name=f"qk{jq}{t}")
                            ps = pss[t]
                            Wt, xt = chains[ch]
                            for kp in range(KP):
                                nc.tensor.matmul(
                                    ps[:],
                                    Wt[:, kp, :, t, :],
                                    xt[:, kp, :, :],
                                    start=(ch == 0 and kp == 0),
                                    stop=(ch == 2 and kp == KP - 1),
                                    perf_mode=DR,
                                )
                            pe_note(4 * 107)
                            if ch == 2:
                                nc.vector.tensor_scalar_add(
                                    qkT[t % 2][:, t // 2, tsq:teq],
                                    ps[:],
                                    bqk_sb[:, t : t + 1],
                                )
                        return emit
                    return [unit(t, ch) for t in (0, 2, 1, 3)
                            for ch in range(3)]

                if j == 0:
                    # inline: g0's tiles first, kp-outer so matmuls track
                    # the x8/xr8 chunk DMAs
                    u0 = make_qkproj_units(0, x8q, xr8q)
                    for u_ in u0[:6]:
                        u_()
                    extra_qk0 = u0[6:]
                else:
                    extra_qk0 = []

                # ---- previous quarter's g1 norm chain ----
                for u_ in pending[:2]:
                    u_()
                nextq = (make_qkproj_units(j + 1, *next_x)
                         if j + 1 < NQS else [])
                # must-run-this-quarter fillers (slot-rate paced)
                queue = extra_qk0 + nextq
                # deferrable fillers (budget paced): carried + prev proj
                defq = carry + pending[2:]
                pending = []
                carry = []

                # ---- V units: fp8 DR compensated, token-major out ----
                def make_v_units(q=j, xs_=x8q, xrs_=xr8q):
                    vch = [(xs_, Wv8_sb), (xrs_, Wv8_sb), (xs_, Wvr8_sb)]

                    def unit(blk):
                        def emit():
                            tb = 4 * q + blk
                            vps = mm.tile([128, 256], F32, tag="mm",
                                          name=f"v{q}{blk}")
                            for ch in range(3):
                                xt, Wt = vch[ch]
                                for kp in range(KP):
                                    nc.tensor.matmul(
                                        vps[:],
                                        xt[:, kp, :,
                                           128 * blk : 128 * (blk + 1)],
                                        Wt[:, kp, :, :],
                                        start=(ch == 0 and kp == 0),
                                        stop=(ch == 2 and kp == KP - 1),
                                        perf_mode=DR,
                                    )
                            pe_note(12 * 53)
                            # vaug = vps * 2^-10 + vbias, bf16
                            nc.vector.scalar_tensor_tensor(
                                out=vaug[:, tb, :, 0:64],
                                in0=vps.rearrange("p (h c) -> p h c", c=64),
                                scalar=VSCALE,
                                in1=vb_sb.rearrange("p (h c) -> p h c", c=64),
                                op0=mybir.AluOpType.mult,
                                op1=mybir.AluOpType.add,
                            )
                        return emit
                    return [unit(blk) for blk in range(4)]

                v_units = make_v_units()

                # ---- attention for q_super j, per head pair g ----
                for g in range(2):
                    o_ps = {
                        half: opool.tile([128, 2, 2, 65], F32, tag="o",
                                         name=f"o{j}{g}{half}")
                        for half in range(2)
                    }
                    first_touch = {half: True for half in range(2)}
                    ets = {}

                    def emit_qk(i, g=g, j=j, ets=ets):
                        t = i - 4 * j
                        qs0 = 128 * t if t >= 0 else 0
                        sps = spool.tile([128, 2, QS], F32, tag="s",
                                         name=f"s{j}{g}{i}")
                        for l in range(2):
                            nc.tensor.matmul(
                                sps[:, l, qs0:],
                                qkT[g][64 * l : 64 * (l + 1), 1,
                                       128 * i : 128 * (i + 1)],
                                qkT[g][64 * l : 64 * (l + 1), 0,
                                       QS * j + qs0 : QS * (j + 1)],
                                start=True,
                                stop=True,
                            )
                        et = et_pool.tile([128, 2, QS], BF16, tag="et",
                                          name=f"et{j}{g}{i}")
                        nc.scalar.activation(
                            et[:, :, qs0:], sps[:, :, qs0:], Exp,
                            scale=EXP_SCALE,
                        )
                        cols = QS - qs0
                        bal["act"] += 2 * cols * 0.8333 + 217
                        bal["pe"] += 2 * cols * 0.4167
                        if t >= 0:
                            # mask the diagonal 128-block on gpsimd
                            nc.gpsimd.affine_select(
                                out=et[:, :, qs0 : qs0 + 128],
                                in_=et[:, :, qs0 : qs0 + 128],
                                compare_op=mybir.AluOpType.is_ge,
                                fill=0.0,
                                base=0,
                                pattern=[[0, 2], [1, 128]],
                                channel_multiplier=-1,
                            )
                        ets[i] = et

                    def emit_av(i, g=g, j=j, ets=ets, o_ps=o_ps,
                                first_touch=first_touch):
                        t = i - 4 * j
                        et = ets.pop(i)
                        for u in range(max(0, t), 4):
                            half, u2 = divmod(u, 2)
                            for l in range(2):
                                st = first_touch[half]
                                first_touch[half] = False
                                nc.tensor.matmul(
                                    o_ps[half][:, l, u2, :],
                                    et[:, l, 128 * u : 128 * (u + 1)],
                                    vaug[:, i, 2 * g + l, :],
                                    start=st,
                                    stop=(i == 4 * j + u),
                                    skip_group_check=True,
                                )
                        bal["pe"] += (4 - max(0, t)) * 2 * 27

                    LOOKAHEAD = 3
                    for i in range(n_i):
                        t = i - 4 * j
                        if i >= LOOKAHEAD:
                            emit_av(i - LOOKAHEAD)
                        if g == 0 and t >= 0:
                            # diagonal AV(i) consumes vaug[4j+t]: flush V
                            while len(v_units) > 3 - t:
                                v_units.pop(0)()
                        if g == 0 and i < 4 and v_units:
                            v_units.pop(0)()
                        # must-queue at slot rate
                        left = (2 - g) * n_i - i - 1
                        quota = (-(-len(queue) // max(1, left))
                                 if left else len(queue))
                        for _ in range(min(quota, 2, len(queue))):
                            queue.pop(0)()
                        # deferrables only into ACT-idle budget
                        while defq and bal["pe"] + 300 < bal["act"]:
                            pe_note(defq.pop(0)() or 0)
                        emit_qk(i)
                    for i in range(max(0, n_i - LOOKAHEAD), n_i):
                        emit_av(i)

                    # ---- deferred normalize + transpose chain for (j, g) ----
                    def make_norm(j=j, g=g, o_ps=o_ps):
                        onorm = {}

                        def norm():
                            on = onorm_pool.tile([128, 4, 2, 64], BF16,
                                                 tag="onorm", name=f"on{j}{g}")
                            onorm[0] = on
                            for half in range(2):
                                rc = work.tile([128, 2, 2], F32, tag="recip",
                                               name=f"rc{j}{g}{half}")
                                nc.vector.reciprocal(
                                    rc[:], o_ps[half][:, :, :, 64])
                                nc.vector.tensor_mul(
                                    on[:, 2 * half : 2 * half + 2, :, :],
                                    o_ps[half][:, :, :, 0:64]
                                        .rearrange("p s u c -> p u s c"),
                                    rc.rearrange("p s u -> p u s")
                                        .unsqueeze(3)
                                        .broadcast_to([128, 2, 2, 64]),
                                )

                        def transp():
                            trp = mm.tile([128, 4, 128], BF16, tag="mm",
                                          name=f"tr{j}{g}")
                            on = onorm[0]
                            for u in range(4):
                                nc.tensor.matmul(
                                    trp[:, u, :],
                                    on[:, u, :, :],
                                    ident[:],
                                    start=True,
                                    stop=True,
                                    is_transpose=True,
                                )
                            pe_note(4 * 53)
                            nc.vector.tensor_copy(
                                onT[:, g, QS * j : QS * (j + 1)],
                                trp.rearrange("p u q -> p (u q)"),
                            )

                        return [norm, transp]

                    if g == 0:
                        # run in g1's segment so the o slots recycle promptly
                        queue = make_norm() + queue
                    else:
                        pending.extend(make_norm())

                # ---- output projection units for quarter j (deferrable,
                # one matmul per unit) ----
                def make_proj(jj=j):
                    tail = jj == NQS - 1
                    ysbs = {}
                    ypss = {}

                    def unit(blk, nh, c):
                        def emit():
                            tb = 4 * jj + blk
                            if c == 0:
                                ypss[(tb, nh)] = mm.tile(
                                    [128, QS], F32, tag="mm",
                                    name=f"y{tb}{nh}")
                            yps = ypss[(tb, nh)]
                            nc.tensor.matmul(
                                yps[:],
                                onT[:, c, 128 * tb : 128 * (tb + 1)],
                                Wp_sb[:, c, QS * nh : QS * (nh + 1)],
                                start=(c == 0),
                                stop=(c == 1),
                            )
                            if c == 0:
                                return 213
                            if nh == 0:
                                ysbs[tb] = ysb_pool.tile(
                                    [128, 2, QS], F32, tag="ysb", bufs=4,
                                    name=f"ysb{tb}")
                            ysb = ysbs[tb]
                            if tail and (blk + nh) % 2 == 1:
                                nc.scalar.copy(ysb[:, nh, :], yps[:])
                            else:
                                nc.vector.tensor_copy(ysb[:, nh, :], yps[:])
                            if nh == 1:
                                nc.sync.dma_start(
                                    y_r[tb], ysb.rearrange("p a b -> p (a b)"))
                            return 213
                        return emit
                    return unit

                # flush must-fillers before the next quarter
                while queue:
                    queue.pop(0)()
                # defq leftovers carry forward
                carry = defq

                pu = make_proj()
                pending.extend(pu(blk, nh, c) for blk in range(4)
                               for nh in range(2) for c in range(2))

            # ---- tail: last quarter's norm chain + remaining projection ----
            for f in pending[:2]:
                f()
            for f in carry:
                f()
            for f in pending[2:]:
                f()

    nc.compile()
    return nc


def _host_prep(x, W_qkv, b_qkv, W_proj, b_proj):
    """Build per-core input maps."""
    import ml_dtypes
    f8 = ml_dtypes.float8_e4m3
    bf = ml_dtypes.bfloat16

    x = np.asarray(x, dtype=np.float32)
    W_qkv = np.asarray(W_qkv, dtype=np.float32)
    b_qkv = np.asarray(b_qkv, dtype=np.float32)
    W_proj = np.asarray(W_proj, dtype=np.float32)

    x8s, xr8s = [], []
    for b in range(B):
        x4 = np.ascontiguousarray(x[b].T) * XS
        x8 = x4.astype(f8)
        xr8 = (x4 - x8.astype(np.float32)).astype(f8)
        x8s.append(x8)
        xr8s.append(xr8)
    ident = np.eye(128, dtype=np.float32).astype(bf)

    def pack_w(Wcols):
        """[1024, F] -> [128, KP, 2, F] (rows chunked 128*(2kp+two)+p)."""
        F = Wcols.shape[1]
        return Wcols.reshape(KP, 2, 128, F).transpose(2, 0, 1, 3)

    in_maps = []
    for c in range(NCORES):
        b, g4 = divmod(c, GROUPS)
        col0 = 256 * g4

        W8 = np.zeros((128, KP, 2, 4, 128), dtype=np.float32)
        Wr8 = np.zeros((128, KP, 2, 4, 128), dtype=np.float32)
        bqk = np.zeros((128, 4), dtype=np.float32)
        for t in range(4):
            qk, g = divmod(t, 2)
            c0 = 1024 * qk + col0 + 128 * g
            Ws = WS * W_qkv[:, c0 : c0 + 128]
            W8t = Ws.astype(f8).astype(np.float32)
            Wr8t = (Ws - W8t).astype(f8).astype(np.float32)
            W8[:, :, :, t, :] = pack_w(W8t)
            Wr8[:, :, :, t, :] = pack_w(Wr8t)
            bqk[:, t] = (XS * WS) * b_qkv[c0 : c0 + 128]
        W8 = np.ascontiguousarray(W8.reshape(128, -1)).astype(f8)
        Wr8 = np.ascontiguousarray(Wr8.reshape(128, -1)).astype(f8)

        Wvs = WS * W_qkv[:, 2048 + col0 : 2048 + col0 + 256]
        Wv8t = Wvs.astype(f8).astype(np.float32)
        Wvr8t = (Wvs - Wv8t).astype(f8).astype(np.float32)
        Wv8 = np.ascontiguousarray(pack_w(Wv8t).reshape(128, -1)).astype(f8)
        Wvr8 = np.ascontiguousarray(pack_w(Wvr8t).reshape(128, -1)).astype(f8)

        bv = b_qkv[2048 + col0 : 2048 + col0 + 256]
        vbias = np.ascontiguousarray(
            np.broadcast_to(bv, (128, 256))).astype(np.float32)
        Wp = np.ascontiguousarray(
            W_proj[col0 : col0 + 256].reshape(2, 128, D).transpose(1, 0, 2)
            .reshape(128, 2 * D)
        ).astype(bf)
        in_maps.append(
            {
                "x8": x8s[b],
                "xr8": xr8s[b],
                "W8": W8,
                "Wr8": Wr8,
                "Wv8": Wv8,
                "Wvr8": Wvr8,
                "Wp": Wp,
                "bqk": bqk,
                "vbias": vbias,
                "identD": ident,
            }
        )
    return in_maps


def _make_runner(nc):
    """Build the PJRT executable once (mirrors bass2jax.run_bass_via_pjrt)
    so repeated kernel() calls skip re-tracing/compile-cache lookups."""
    import jax
    from jax.sharding import Mesh, PartitionSpec
    from jax.experimental.shard_map import shard_map

    from concourse.bass2jax import (
        _bass_exec_p,
        install_neuronx_cc_hook,
        partition_id_tensor,
    )

    install_neuronx_cc_hook()
    partition_name = (
        nc.partition_id_tensor.name if nc.partition_id_tensor else None
    )
    in_names, out_names, out_avals, zero_outs = [], [], [], []
    for alloc in nc.m.functions[0].allocations:
        if not isinstance(alloc, mybir.MemoryLocationSet):
            continue
        name = alloc.memorylocations[0].name
        if alloc.kind == "ExternalInput":
            if name != partition_name:
                in_names.append(name)
        elif alloc.kind == "ExternalOutput":
            out_names.append(name)
            shape = tuple(alloc.tensor_shape)
            dtype = mybir.dt.np(alloc.dtype)
            out_avals.append(jax.core.ShapedArray(shape, dtype))
            zero_outs.append(np.zeros(shape, dtype))
    n_params = len(in_names)
    all_in_names = in_names + out_names
    if partition_name is not None:
        all_in_names = all_in_names + [partition_name]

    def _body(*args):
        operands = list(args)
        if partition_name is not None:
            operands.append(partition_id_tensor())
        return tuple(
            _bass_exec_p.bind(
                *operands,
                out_avals=tuple(out_avals),
                in_names=tuple(all_in_names),
                out_names=tuple(out_names),
                lowering_input_output_aliases=(),
                sim_require_finite=True,
                sim_require_nnan=True,
                nc=nc,
            )
        )

    devices = jax.devices()[:NCORES]
    mesh = Mesh(np.asarray(devices), ("core",))
    in_specs = (PartitionSpec("core"),) * (n_params + len(out_names))
    out_specs = (PartitionSpec("core"),) * len(out_names)
    fn = jax.jit(
        shard_map(_body, mesh=mesh, in_specs=in_specs,
                  out_specs=out_specs, check_rep=False),
        keep_unused=True,
    )
    concat_zeros = [
        np.zeros((NCORES * z.shape[0], *z.shape[1:]), z.dtype)
        for z in zero_outs
    ]

    def run(in_maps):
        concat_in = [
            np.concatenate([np.asarray(m[name]) for m in in_maps], axis=0)
            for name in in_names
        ]
        out_arrs = fn(*concat_in, *concat_zeros)
        return [
            {
                name: np.asarray(out_arrs[i]).reshape(
                    NCORES, *out_avals[i].shape
                )[c]
                for i, name in enumerate(out_names)
            }
            for c in range(NCORES)
        ]

    return run


def kernel(x, W_qkv, b_qkv, W_proj, b_proj):
    if "nc" not in _CACHE:
        _CACHE["nc"] = _build()
        try:
            _CACHE["run"] = _make_runner(_CACHE["nc"])
        except Exception:
            _CACHE["run"] = None
    in_maps = _host_prep(x, W_qkv, b_qkv, W_proj, b_proj)
    results = None
    if _CACHE["run"] is not None:
        try:
            results = _CACHE["run"](in_maps)
        except Exception:
            results = None
    if results is None:
        results = run_bass_kernel_spmd(
            _CACHE["nc"], in_maps, core_ids=list(range(NCORES))
        ).results
    out = np.zeros((B, N, D), dtype=np.float32)
    bp = np.asarray(b_proj, dtype=np.float32)
    for b in range(B):
        acc = results[4 * b]["y"].astype(np.float32).copy()
        for g in range(1, GROUPS):
            acc += results[4 * b + g]["y"]
        out[b] = acc + bp
    return out


# revision 26
# speedup vs baseline: 1.2323x; 1.0018x over previous
"""Causal self-attention (b=2, n=2048, d=1024, 16 heads) on 8 NeuronCores.

Sharding: core c handles batch b = c // 4 and head group g4 = c % 4
(heads 4*g4..4*g4+3).  qkv weights column-sharded, proj weights row-sharded
(Megatron); each core emits a partial [2048, 1024] proj output and the
host sums the 4 partials per batch (b_proj added host-side).

Engine plan (per core, cost-model driven):
  q/k/v projection : fp8e4m3 DoubleRow with residual compensation:
                     x -> f8(4x) + f8(residual), W -> f8(256W) +
                     f8(residual); acc = x8@W8 + xr8@W8 + x8@Wr8 in one
                     PSUM group (all terms share the 1024x scale), giving
                     ~0.2% error at 1.5x the fp8 matmul cost.
  S = K^T Q        : bf16 (q/k fp8 would put ~1.5%/quantization of
                     correlated tilt into softmax - too close to the 2e-2
                     gate), heads at partition bases 0/64, exact causal
                     triangle at 128-col granularity; exp folds the 1/8
                     softmax scale and the 2^-20 projection scale.
  exp              : ACT, PSUM f32 -> SBUF bf16.
  causal mask      : gpsimd affine_select on the diagonal et blocks.
  A @ V            : flipped: et [128k,128q] stationary (bf16), [V|1]
                     [128k,65] moving -> token-major o [128q,65] with the
                     softmax denominator in column 64.  Four 65-col
                     accumulation regions share one PSUM bank: only the
                     first matmul into the bank uses start=True (bank-wide
                     pending-zero), later regions' first matmul overwrites
                     via the pending-zero bits.
  normalize        : DVE reciprocal of col 64 + per-(token,head) multiply,
                     output bf16 token-major.
  transpose        : PE is_transpose (bf16, identity rhs) back to
                     feature-major onT for the output projection.
  projection       : bf16 onT stationary x bf16 Wp moving; f32 y out.

Scheduling: PE work (~92us) exceeds ACT exp (~77us), so QK->exp supply is
first-class; V/qkproj(j+1)/norm units drain at a fixed slot rate while
deferrable proj units pop only when the quarter's emitted exp time exceeds
its emitted PE time (leftovers carry to the next quarter / tail).
"""
import sys

sys.path.insert(0, "/opt/trn_rl_repo")

import numpy as np

import concourse.bass as bass  # noqa: F401
import concourse.mybir as mybir
import concourse.tile as tile
from concourse import bacc
from concourse.bass_utils import run_bass_kernel_spmd

F32 = mybir.dt.float32
F32R = mybir.dt.float32r
BF16 = mybir.dt.bfloat16
FP8 = mybir.dt.float8e4
Exp = mybir.ActivationFunctionType.Exp
DR = mybir.MatmulPerfMode.DoubleRow

B = 2
N = 2048
D = 1024
NH = 16
HD = 64
NCORES = 8
GROUPS = 4                # head groups (cores per batch)
HPC = NH // GROUPS        # heads per core = 4
QS = 512                  # q_super width
NQS = N // QS             # 4
NB = N // 128             # 16 token blocks
CCH = D // 128            # 8 contraction chunks
KP = CCH // 2             # 4 DoubleRow contraction pairs
XS = 4.0                  # fp8 scale on x
WS = 256.0                # fp8 scale on W_qkv
EXP_SCALE = 0.125 / (XS * WS) ** 2     # q,k both carry the 1024x scale
VSCALE = 1.0 / (XS * WS)               # V drain rescale

_CACHE = {}


def _build():
    nc = bacc.Bacc("TRN2", target_bir_lowering=False, debug=False,
                   num_devices=NCORES)
    x8d = nc.dram_tensor("x8", [D, N], FP8, kind="ExternalInput").ap()
    xr8d = nc.dram_tensor("xr8", [D, N], FP8, kind="ExternalInput").ap()
    W8d = nc.dram_tensor("W8", [128, KP * 2 * 4 * 128], FP8,
                         kind="ExternalInput").ap()
    Wr8d = nc.dram_tensor("Wr8", [128, KP * 2 * 4 * 128], FP8,
                          kind="ExternalInput").ap()
    Wv8d = nc.dram_tensor("Wv8", [128, KP * 2 * 256], FP8,
                          kind="ExternalInput").ap()
    Wvr8d = nc.dram_tensor("Wvr8", [128, KP * 2 * 256], FP8,
                           kind="ExternalInput").ap()
    Wp = nc.dram_tensor("Wp", [128, 2 * D], BF16, kind="ExternalInput").ap()
    bqk = nc.dram_tensor("bqk", [128, 4], F32, kind="ExternalInput").ap()
    vbias = nc.dram_tensor("vbias", [128, 256], F32, kind="ExternalInput").ap()
    identD = nc.dram_tensor("identD", [128, 128], BF16, kind="ExternalInput").ap()
    y = nc.dram_tensor("y", [N, D], BF16, kind="ExternalOutput").ap()

    with tile.TileContext(nc) as tc:
        with (
            tc.tile_pool(name="persist", bufs=1) as pp,
            tc.tile_pool(name="x8_pool", bufs=2) as x8_pool,
            tc.tile_pool(name="xr8_pool", bufs=2) as xr8_pool,
            tc.tile_pool(name="et_pool", bufs=6) as et_pool,
            tc.tile_pool(name="onorm_pool", bufs=2) as onorm_pool,
            tc.tile_pool(name="work", bufs=4) as work,
            tc.tile_pool(name="ysb_pool", bufs=4) as ysb_pool,
            tc.tile_pool(name="mm", bufs=2, space="PSUM") as mm,
            tc.tile_pool(name="spool", bufs=2, space="PSUM") as spool,
            tc.tile_pool(name="opool", bufs=2, space="PSUM") as opool,
        ):
            # ---- persistent tiles ----
            W8_sb = pp.tile([128, KP, 2, 4, 128], FP8)    # (kp, two, tile, m)
            Wr8_sb = pp.tile([128, KP, 2, 4, 128], FP8)
            Wv8_sb = pp.tile([128, KP, 2, 256], FP8)
            Wvr8_sb = pp.tile([128, KP, 2, 256], FP8)
            Wp_sb = pp.tile([128, 2, D], BF16)
            bqk_sb = pp.tile([128, 4], F32)               # per (qk,g) tile
            vb_sb = pp.tile([128, 256], F32)
            ident = pp.tile([128, 128], BF16)
            # q/k bf16 (1024x scaled), heads paired at bases 0/64
            qkT = {g: pp.tile([128, 2, N], BF16, name=f"qkT_{g}")
                   for g in range(2)}
            vaug = pp.tile([128, NB, HPC, 65], BF16)      # [V | 1] token-major
            onT = pp.tile([128, 2, N], BF16)              # feature-major o

            W8_r = W8d.rearrange("p (kp two t m) -> p kp two t m",
                                 kp=KP, two=2, t=4)
            Wr8_r = Wr8d.rearrange("p (kp two t m) -> p kp two t m",
                                   kp=KP, two=2, t=4)
            Wv8_r = Wv8d.rearrange("p (kp two f) -> p kp two f", kp=KP, two=2)
            Wvr8_r = Wvr8d.rearrange("p (kp two f) -> p kp two f",
                                     kp=KP, two=2)
            x8_r = x8d.rearrange("(kp two p) n -> p kp two n", p=128, two=2)
            xr8_r = xr8d.rearrange("(kp two p) n -> p kp two n", p=128, two=2)
            Wp_r = Wp.rearrange("p (c f) -> p c f", c=2)
            y_r = y.rearrange("(t p) f -> t p f", p=128)

            def fetch_x(q):
                t0, t1 = QS * q, QS * (q + 1)
                x8q = x8_pool.tile([128, KP, 2, QS], FP8, tag="x8",
                                   name=f"x8_{q}")
                nc.sync.dma_start(x8q[:], x8_r[:, :, :, t0:t1])
                xr8q = xr8_pool.tile([128, KP, 2, QS], FP8, tag="xr8",
                                     name=f"xr8_{q}")
                nc.sync.dma_start(xr8q[:], xr8_r[:, :, :, t0:t1])
                return x8q, xr8q

            # ---------- per-quarter state ----------
            onorm_sh = {}      # (j, g) -> onorm tile (shared across halves)
            pending = []       # from quarter j-1: [norm_g1, transp_g1] + proj
            carry = []         # deferrable units carried across quarters
            next_x = None
            # pacing state (reset per quarter): ns of exp emitted vs ns of
            # PE emitted
            bal = {"act": 0.0, "pe": 0.0}

            def pe_note(ns):
                bal["pe"] += ns

            for j in range(NQS):
                ts, te = QS * j, QS * (j + 1)
                n_i = 4 * j + 4
                bal["act"] = 0.0
                bal["pe"] = 0.0

                # ---- input DMAs, ordered by first consumption ----
                if j == 0:
                    # quarter-0 critical path: W8/Wr8 first, then per-kp
                    # x8/xr8 chunks so qkproj tracks chunk arrivals
                    nc.sync.dma_start(W8_sb[:], W8_r)
                    nc.sync.dma_start(Wr8_sb[:], Wr8_r)
                    nc.sync.dma_start(bqk_sb[:], bqk)
                    x8q = x8_pool.tile([128, KP, 2, QS], FP8, tag="x8",
                                       name="x8_0")
                    xr8q = xr8_pool.tile([128, KP, 2, QS], FP8, tag="xr8",
                                         name="xr8_0")
                    for kp in range(KP):
                        nc.sync.dma_start(x8q[:, kp, :, :],
                                          x8_r[:, kp, :, 0:QS])
                        nc.sync.dma_start(xr8q[:, kp, :, :],
                                          xr8_r[:, kp, :, 0:QS])
                    nc.sync.dma_start(Wv8_sb[:], Wv8_r)
                    nc.sync.dma_start(Wvr8_sb[:], Wvr8_r)
                    nc.sync.dma_start(vb_sb[:], vbias)
                    nc.sync.dma_start(ident[:], identD)
                    nc.sync.dma_start(Wp_sb[:], Wp_r)
                    # ones columns of [V|1] via DVE (0*x + 1)
                    nc.vector.tensor_scalar(
                        out=vaug[:, :, :, 64],
                        in0=ident[:, 0:64].rearrange("p (a b) -> p a b", b=HPC),
                        scalar1=0.0,
                        scalar2=1.0,
                        op0=mybir.AluOpType.mult,
                        op1=mybir.AluOpType.add,
                    )
                    next_x = fetch_x(1)
                else:
                    x8q, xr8q = next_x
                    if j + 1 < NQS:
                        next_x = fetch_x(j + 1)

                # ---- q/k projection: tile t = 2*qk + g = 128 features of
                # heads {2g, 2g+1}; three DoubleRow chains share one PSUM
                # accumulation (all at the 1024x scale).
                def make_qkproj_units(jq, xs_, xrs_):
                    tsq, teq = QS * jq, QS * (jq + 1)
                    pss = {}
                    chains = [(W8_sb, xs_), (W8_sb, xrs_), (Wr8_sb, xs_)]

                    def unit(t, ch):
                        def emit():
                            if ch == 0:
                                pss[t] = mm.tile([128, QS], F32, tag="mm",
                                                 name=f"qk{jq}{t}")
                            ps = pss[t]
                            Wt, xt = chains[ch]
                            for kp in range(KP):
                                nc.tensor.matmul(
                                    ps[:],
                                    Wt[:, kp, :, t, :],
                                    xt[:, kp, :, :],
                                    start=(ch == 0 and kp == 0),
                                    stop=(ch == 2 and kp == KP - 1),
                                    perf_mode=DR,
                                )
                            pe_note(4 * 107)
                            if ch == 2:
                                nc.vector.tensor_scalar_add(
                                    qkT[t % 2][:, t // 2, tsq:teq],
                                    ps[:],
                                    bqk_sb[:, t : t + 1],
                                )
                        return emit
                    return [unit(t, ch) for t in (0, 2, 1, 3)
                            for ch in range(3)]

                if j == 0:
                    # inline: g0's tiles first, kp-outer so matmuls track
                    # the x8/xr8 chunk DMAs
                    u0 = make_qkproj_units(0, x8q, xr8q)
                    for u_ in u0[:6]:
                        u_()
                    extra_qk0 = u0[6:]
                else:
                    extra_qk0 = []

                # ---- previous quarter's g1 norm chain ----
                for u_ in pending[:2]:
                    u_()
                nextq = (make_qkproj_units(j + 1, *next_x)
                         if j + 1 < NQS else [])
                # must-run-this-quarter fillers (slot-rate paced)
                queue = extra_qk0 + nextq
                # deferrable fillers (budget paced): carried + prev proj
                defq = carry + pending[2:]
                pending = []
                carry = []

                # ---- V units: fp8 DR compensated, token-major out ----
                def make_v_units(q=j, xs_=x8q, xrs_=xr8q):
                    vch = [(xs_, Wv8_sb), (xrs_, Wv8_sb), (xs_, Wvr8_sb)]

                    def unit(blk):
                        def emit():
                            tb = 4 * q + blk
                            vps = mm.tile([128, 256], F32, tag="mm",
                                          name=f"v{q}{blk}")
                            for ch in range(3):
                                xt, Wt = vch[ch]
                                for kp in range(KP):
                                    nc.tensor.matmul(
                                        vps[:],
                                        xt[:, kp, :,
                                           128 * blk : 128 * (blk + 1)],
                                        Wt[:, kp, :, :],
                                        start=(ch == 0 and kp == 0),
                                        stop=(ch == 2 and kp == KP - 1),
                                        perf_mode=DR,
                                    )
                            pe_note(12 * 53)
                            # vaug = vps * 2^-10 + vbias, bf16
                            nc.vector.scalar_tensor_tensor(
                                out=vaug[:, tb, :, 0:64],
                                in0=vps.rearrange("p (h c) -> p h c", c=64),
                                scalar=VSCALE,
                                in1=vb_sb.rearrange("p (h c) -> p h c", c=64),
                                op0=mybir.AluOpType.mult,
                                op1=mybir.AluOpType.add,
                            )
                        return emit
                    return [unit(blk) for blk in range(4)]

                # ---- output projection units for quarter j (deferrable,
                # one matmul per unit) ----
                def make_proj(jj=j):
                    tail = jj == NQS - 1
                    ysbs = {}
                    ypss = {}

                    def unit(blk, nh, c):
                        def emit():
                            tb = 4 * jj + blk
                            if c == 0:
                                ypss[(tb, nh)] = mm.tile(
                                    [128, QS], F32, tag="mm",
                                    name=f"y{tb}{nh}")
                            yps = ypss[(tb, nh)]
                            nc.tensor.matmul(
                                yps[:],
                                onT[:, c, 128 * tb : 128 * (tb + 1)],
                                Wp_sb[:, c, QS * nh : QS * (nh + 1)],
                                start=(c == 0),
                                stop=(c == 1),
                            )
                            if c == 0:
                                return 213
                            if nh == 0:
                                ysbs[tb] = ysb_pool.tile(
                                    [128, 2, QS], BF16, tag="ysb", bufs=4,
                                    name=f"ysb{tb}")
                            ysb = ysbs[tb]
                            if tail and (blk + nh) % 2 == 1:
                                nc.scalar.copy(ysb[:, nh, :], yps[:])
                            else:
                                nc.vector.tensor_copy(ysb[:, nh, :], yps[:])
                            if nh == 1:
                                nc.sync.dma_start(
                                    y_r[tb], ysb.rearrange("p a b -> p (a b)"))
                            return 213
                        return emit
                    return unit

                pu = make_proj()

                v_units = make_v_units()

                # ---- attention for q_super j, per head pair g ----
                for g in range(2):
                    o_ps = {
                        half: opool.tile([128, 2, 2, 65], F32, tag="o",
                                         name=f"o{j}{g}{half}")
                        for half in range(2)
                    }
                    first_touch = {half: True for half in range(2)}
                    ets = {}

                    def emit_qk(i, g=g, j=j, ets=ets):
                        t = i - 4 * j
                        qs0 = 128 * t if t >= 0 else 0
                        sps = spool.tile([128, 2, QS], F32, tag="s",
                                         name=f"s{j}{g}{i}")
                        for l in range(2):
                            nc.tensor.matmul(
                                sps[:, l, qs0:],
                                qkT[g][64 * l : 64 * (l + 1), 1,
                                       128 * i : 128 * (i + 1)],
                                qkT[g][64 * l : 64 * (l + 1), 0,
                                       QS * j + qs0 : QS * (j + 1)],
                                start=True,
                                stop=True,
                            )
                        et = et_pool.tile([128, 2, QS], BF16, tag="et",
                                          name=f"et{j}{g}{i}")
                        nc.scalar.activation(
                            et[:, :, qs0:], sps[:, :, qs0:], Exp,
                            scale=EXP_SCALE,
                        )
                        cols = QS - qs0
                        bal["act"] += 2 * cols * 0.8333 + 217
                        bal["pe"] += 2 * cols * 0.4167
                        if t >= 0:
                            # mask the diagonal 128-block on gpsimd
                            nc.gpsimd.affine_select(
                                out=et[:, :, qs0 : qs0 + 128],
                                in_=et[:, :, qs0 : qs0 + 128],
                                compare_op=mybir.AluOpType.is_ge,
                                fill=0.0,
                                base=0,
                                pattern=[[0, 2], [1, 128]],
                                channel_multiplier=-1,
                            )
                        ets[i] = et

                    def emit_av(i, g=g, j=j, ets=ets, o_ps=o_ps,
                                first_touch=first_touch):
                        t = i - 4 * j
                        if g == 0 and t >= 0:
                            # diagonal AV(i) consumes vaug[4j+t]: flush V
                            while len(v_units) > 3 - t:
                                v_units.pop(0)()
                        et = ets.pop(i)
                        for u in range(max(0, t), 4):
                            half, u2 = divmod(u, 2)
                            for l in range(2):
                                st = first_touch[half]
                                first_touch[half] = False
                                nc.tensor.matmul(
                                    o_ps[half][:, l, u2, :],
                                    et[:, l, 128 * u : 128 * (u + 1)],
                                    vaug[:, i, 2 * g + l, :],
                                    start=st,
                                    stop=(i == 4 * j + u),
                                    skip_group_check=True,
                                )
                        bal["pe"] += (4 - max(0, t)) * 2 * 27

                    LOOKAHEAD = 3
                    for i in range(n_i):
                        if i >= LOOKAHEAD:
                            emit_av(i - LOOKAHEAD)
                        if g == 0 and i < 4 and v_units:
                            v_units.pop(0)()
                        # must-queue at slot rate
                        left = (2 - g) * n_i - i - 1
                        quota = (-(-len(queue) // max(1, left))
                                 if left else len(queue))
                        for _ in range(min(quota, 2, len(queue))):
                            queue.pop(0)()
                        # deferrables only into ACT-idle budget
                        while defq and bal["pe"] + 300 < bal["act"]:
                            pe_note(defq.pop(0)() or 0)
                        emit_qk(i)
                    # ---- deferred normalize + transpose chain for (j, g);
                    # halves=(0,), (1,) or (0, 1) per unit pair ----
                    def make_norm(j=j, g=g, o_ps=o_ps, halves=(0, 1)):
                        onorm = {}

                        def norm():
                            key = (j, g)
                            if key not in onorm_sh:
                                onorm_sh[key] = onorm_pool.tile(
                                    [128, 4, 2, 64], BF16, tag="onorm",
                                    name=f"on{j}{g}")
                            on = onorm_sh[key]
                            onorm[0] = on
                            for half in halves:
                                rc = work.tile([128, 2, 2], F32, tag="recip",
                                               name=f"rc{j}{g}{half}")
                                nc.vector.reciprocal(
                                    rc[:], o_ps[half][:, :, :, 64])
                                nc.vector.tensor_mul(
                                    on[:, 2 * half : 2 * half + 2, :, :],
                                    o_ps[half][:, :, :, 0:64]
                                        .rearrange("p s u c -> p u s c"),
                                    rc.rearrange("p s u -> p u s")
                                        .unsqueeze(3)
                                        .broadcast_to([128, 2, 2, 64]),
                                )

                        def transp():
                            on = onorm[0]
                            for half in halves:
                                trp_h = mm.tile(
                                    [128, 2, 128], BF16, tag="mm",
                                    name=f"tr{j}{g}{half}")
                                for uu in range(2):
                                    u = 2 * half + uu
                                    nc.tensor.matmul(
                                        trp_h[:, uu, :],
                                        on[:, u, :, :],
                                        ident[:],
                                        start=True,
                                        stop=True,
                                        is_transpose=True,
                                    )
                                pe_note(2 * 53)
                                nc.vector.tensor_copy(
                                    onT[:, g, QS * j + 256 * half :
                                        QS * j + 256 * half + 256],
                                    trp_h.rearrange("p u q -> p (u q)"),
                                )

                        return [norm, transp]

                    if j == NQS - 1 and g == 1:
                        # tail: per-half chains interleave into the drain so
                        # the projection starts before the last exps finish
                        emit_av(n_i - LOOKAHEAD)      # stops u0, u1
                        for f in make_norm(halves=(0,)):
                            f()
                        for blk in (0, 1):
                            for nh in range(2):
                                for c in range(2):
                                    pu(blk, nh, c)()
                        emit_av(n_i - 2)
                        emit_av(n_i - 1)
                        for f in make_norm(halves=(1,)):
                            f()
                        for blk in (2, 3):
                            for nh in range(2):
                                for c in range(2):
                                    pu(blk, nh, c)()
                    else:
                        for i in range(max(0, n_i - LOOKAHEAD), n_i):
                            emit_av(i)
                        if g == 0:
                            # run in g1's segment: o slots recycle promptly
                            queue = make_norm() + queue
                        else:
                            pending.extend(make_norm())

                # flush must-fillers before the next quarter
                while queue:
                    queue.pop(0)()
                # defq leftovers carry forward
                carry = defq
                if j < NQS - 1:
                    pending.extend(pu(blk, nh, c) for blk in range(4)
                                   for nh in range(2) for c in range(2))



            # ---- tail: any carried deferrable units ----
            for f in carry:
                f()
            for f in pending:
                f()

    nc.compile()
    return nc


def _host_prep(x, W_qkv, b_qkv, W_proj, b_proj):
    """Build per-core input maps."""
    import ml_dtypes
    f8 = ml_dtypes.float8_e4m3
    bf = ml_dtypes.bfloat16

    x = np.asarray(x, dtype=np.float32)
    W_qkv = np.asarray(W_qkv, dtype=np.float32)
    b_qkv = np.asarray(b_qkv, dtype=np.float32)
    W_proj = np.asarray(W_proj, dtype=np.float32)

    x8s, xr8s = [], []
    for b in range(B):
        x4 = np.ascontiguousarray(x[b].T) * XS
        x8 = x4.astype(f8)
        xr8 = (x4 - x8.astype(np.float32)).astype(f8)
        x8s.append(x8)
        xr8s.append(xr8)
    ident = np.eye(128, dtype=np.float32).astype(bf)

    def pack_w(Wcols):
        """[1024, F] -> [128, KP, 2, F] (rows chunked 128*(2kp+two)+p)."""
        F = Wcols.shape[1]
        return Wcols.reshape(KP, 2, 128, F).transpose(2, 0, 1, 3)

    in_maps = []
    for c in range(NCORES):
        b, g4 = divmod(c, GROUPS)
        col0 = 256 * g4

        W8 = np.zeros((128, KP, 2, 4, 128), dtype=np.float32)
        Wr8 = np.zeros((128, KP, 2, 4, 128), dtype=np.float32)
        bqk = np.zeros((128, 4), dtype=np.float32)
        for t in range(4):
            qk, g = divmod(t, 2)
            c0 = 1024 * qk + col0 + 128 * g
            Ws = WS * W_qkv[:, c0 : c0 + 128]
            W8t = Ws.astype(f8).astype(np.float32)
            Wr8t = (Ws - W8t).astype(f8).astype(np.float32)
            W8[:, :, :, t, :] = pack_w(W8t)
            Wr8[:, :, :, t, :] = pack_w(Wr8t)
            bqk[:, t] = (XS * WS) * b_qkv[c0 : c0 + 128]
        W8 = np.ascontiguousarray(W8.reshape(128, -1)).astype(f8)
        Wr8 = np.ascontiguousarray(Wr8.reshape(128, -1)).astype(f8)

        Wvs = WS * W_qkv[:, 2048 + col0 : 2048 + col0 + 256]
        Wv8t = Wvs.astype(f8).astype(np.float32)
        Wvr8t = (Wvs - Wv8t).astype(f8).astype(np.float32)
        Wv8 = np.ascontiguousarray(pack_w(Wv8t).reshape(128, -1)).astype(f8)
        Wvr8 = np.ascontiguousarray(pack_w(Wvr8t).reshape(128, -1)).astype(f8)

        bv = b_qkv[2048 + col0 : 2048 + col0 + 256]
        vbias = np.ascontiguousarray(
            np.broadcast_to(bv, (128, 256))).astype(np.float32)
        Wp = np.ascontiguousarray(
            W_proj[col0 : col0 + 256].reshape(2, 128, D).transpose(1, 0, 2)
            .reshape(128, 2 * D)
        ).astype(bf)
        in_maps.append(
            {
                "x8": x8s[b],
                "xr8": xr8s[b],
                "W8": W8,
                "Wr8": Wr8,
                "Wv8": Wv8,
                "Wvr8": Wvr8,
                "Wp": Wp,
                "bqk": bqk,
                "vbias": vbias,
                "identD": ident,
            }
        )
    return in_maps


def _make_runner(nc):
    """Build the PJRT executable once (mirrors bass2jax.run_bass_via_pjrt)
    so repeated kernel() calls skip re-tracing/compile-cache lookups."""
    import jax
    from jax.sharding import Mesh, PartitionSpec
    from jax.experimental.shard_map import shard_map

    from concourse.bass2jax import (
        _bass_exec_p,
        install_neuronx_cc_hook,
        partition_id_tensor,
    )

    install_neuronx_cc_hook()
    partition_name = (
        nc.partition_id_tensor.name if nc.partition_id_tensor else None
    )
    in_names, out_names, out_avals, zero_outs = [], [], [], []
    for alloc in nc.m.functions[0].allocations:
        if not isinstance(alloc, mybir.MemoryLocationSet):
            continue
        name = alloc.memorylocations[0].name
        if alloc.kind == "ExternalInput":
            if name != partition_name:
                in_names.append(name)
        elif alloc.kind == "ExternalOutput":
            out_names.append(name)
            shape = tuple(alloc.tensor_shape)
            dtype = mybir.dt.np(alloc.dtype)
            out_avals.append(jax.core.ShapedArray(shape, dtype))
            zero_outs.append(np.zeros(shape, dtype))
    n_params = len(in_names)
    all_in_names = in_names + out_names
    if partition_name is not None:
        all_in_names = all_in_names + [partition_name]

    def _body(*args):
        operands = list(args)
        if partition_name is not None:
            operands.append(partition_id_tensor())
        return tuple(
            _bass_exec_p.bind(
                *operands,
                out_avals=tuple(out_avals),
                in_names=tuple(all_in_names),
                out_names=tuple(out_names),
                lowering_input_output_aliases=(),
                sim_require_finite=True,
                sim_require_nnan=True,
                nc=nc,
            )
        )

    devices = jax.devices()[:NCORES]
    mesh = Mesh(np.asarray(devices), ("core",))
    in_specs = (PartitionSpec("core"),) * (n_params + len(out_names))
    out_specs = (PartitionSpec("core"),) * len(out_names)
    fn = jax.jit(
        shard_map(_body, mesh=mesh, in_specs=in_specs,
                  out_specs=out_specs, check_rep=False),
        keep_unused=True,
    )
    concat_zeros = [
        np.zeros((NCORES * z.shape[0], *z.shape[1:]), z.dtype)
        for z in zero_outs
    ]

    def run(in_maps):
        concat_in = [
            np.concatenate([np.asarray(m[name]) for m in in_maps], axis=0)
            for name in in_names
        ]
        out_arrs = fn(*concat_in, *concat_zeros)
        return [
            {
                name: np.asarray(out_arrs[i]).reshape(
                    NCORES, *out_avals[i].shape
                )[c]
                for i, name in enumerate(out_names)
            }
            for c in range(NCORES)
        ]

    return run


def kernel(x, W_qkv, b_qkv, W_proj, b_proj):
    if "nc" not in _CACHE:
        _CACHE["nc"] = _build()
        try:
            _CACHE["run"] = _make_runner(_CACHE["nc"])
        except Exception:
            _CACHE["run"] = None
    in_maps = _host_prep(x, W_qkv, b_qkv, W_proj, b_proj)
    results = None
    if _CACHE["run"] is not None:
        try:
            results = _CACHE["run"](in_maps)
        except Exception:
            results = None
    if results is None:
        results = run_bass_kernel_spmd(
            _CACHE["nc"], in_maps, core_ids=list(range(NCORES))
        ).results
    out = np.zeros((B, N, D), dtype=np.float32)
    bp = np.asarray(b_proj, dtype=np.float32)
    for b in range(B):
        acc = results[4 * b]["y"].astype(np.float32).copy()
        for g in range(1, GROUPS):
            acc += results[4 * b + g]["y"]
        out[b] = acc + bp
    return out
